# revision 7
# baseline (speedup 1.0000x reference)
"""MoE actor (16 experts, top-4) Trainium2 kernel, data-parallel over 8 NeuronCores.

Math per token t:
    logits = x @ router_w.T + router_b             [E]
    probs  = softmax(logits)
    sp     = probs * topk4_mask(logits)            [E]  (masked, not renormalized)
    mean   = sum_e sp[e] * (x @ mean_w[e].T    + mean_b[e])
    lstd   = sum_e sp[e] * (x @ log_std_w[e].T + log_std_b[e])
    lstd   = 1.75 * tanh(lstd) - 3.25

Device strategy (per core, T=2048 tokens):
  - x arrives transposed+bf16 (xT [512, T]); expert weights arrive as one
    concatenated stack wcat[o, e*512+a] (mean|log_std along a, 512 wide).
  - Router: 64 small matmuls -> logits [t,16]; DVE max8 threshold for top-4
    mask; ACT exp with accumulated denominator; sp -> PE-transpose -> spT.
  - spT bounces through DRAM so it can be partition-broadcast-loaded.
  - Main: for each 512-token chunk, for each expert: scale xT tiles by the
    broadcast gate row (DVE), then 16 bf16 matmuls accumulate all experts +
    bias matmul into 4 PSUM banks = outT[512, chunk] (f32).
  - a-rows 256..511 are log_std: tanh (ACT) + affine (DVE) before store.

No collectives: pure SPMD data parallelism; host shards/gathers.
"""

from contextlib import ExitStack

import ml_dtypes
import numpy as np

import concourse.bass as bass
import concourse.mybir as mybir
import concourse.tile as tile
from concourse import bacc
from concourse.bass_utils import run_bass_kernel_spmd
from concourse.masks import make_identity

BF16 = mybir.dt.bfloat16
F32 = mybir.dt.float32
NP_BF16 = ml_dtypes.bfloat16

P = 128
NCORES = 8
B_FULL = 16384
OBS = 512
ACT_DIM = 256
E = 16
A2 = 2 * ACT_DIM  # 512: mean|log_std concatenated
OT = OBS // P     # 4 o-tiles

LOG_STD_SCALE = 3.5   # 0.5*(LOG_STD_MAX-LOG_STD_MIN)
LOG_STD_SHIFT = -1.5  # LOG_STD_MIN + 0.5*(MAX-MIN)


def build_nc(T):
    """Build the single-core Bacc program for a T-token shard."""
    TCH = min(512, T)       # token chunk (psum free dim)
    NTC = T // TCH          # chunks
    NTT = T // P            # router token tiles
    assert T % P == 0 and (T % TCH == 0)
    TILES_PER_CH = TCH // P

    nc = bacc.Bacc("TRN2", target_bir_lowering=False, debug=False)

    xT_d = nc.declare_dram_parameter("xT", [OBS, T], BF16, isOutput=False)
    xTf_d = nc.declare_dram_parameter("xTf", [OBS, T], F32, isOutput=False)
    wcat_d = nc.declare_dram_parameter("wcat", [OBS, E * A2], BF16, isOutput=False)
    bcat_d = nc.declare_dram_parameter("bcat", [E, A2], BF16, isOutput=False)
    rwT_d = nc.declare_dram_parameter("rwT", [OBS, E], F32, isOutput=False)
    rb_d = nc.declare_dram_parameter("rb", [1, E], F32, isOutput=False)
    outT_d = nc.declare_dram_parameter("outT", [A2, T], F32, isOutput=True)

    with tile.TileContext(nc) as tc, ExitStack() as ctx:
        wpool = ctx.enter_context(tc.tile_pool(name="weights", bufs=1))
        dpool = ctx.enter_context(tc.tile_pool(name="spd", bufs=1, space="DRAM"))

        X = []
        for o in range(OT):
            xt = wpool.tile([P, T], BF16, tag=f"x{o}")
            nc.sync.dma_start(xt[:], xT_d[o * P:(o + 1) * P, :])
            X.append(xt)
        Xf = []
        for o in range(OT):
            xft = wpool.tile([P, T], F32, tag=f"xf{o}")
            nc.sync.dma_start(xft[:], xTf_d[o * P:(o + 1) * P, :])
            Xf.append(xft)
        W = []
        for o in range(OT):
            wt = wpool.tile([P, E * A2], BF16, tag=f"w{o}")
            nc.sync.dma_start(wt[:], wcat_d[o * P:(o + 1) * P, :])
            W.append(wt)
        Bc = wpool.tile([E, A2], BF16, tag="bc")
        nc.sync.dma_start(Bc[:], bcat_d[:, :])
        RW = []
        for o in range(OT):
            rwt = wpool.tile([P, E], F32, tag=f"rw{o}")
            nc.sync.dma_start(rwt[:], rwT_d[o * P:(o + 1) * P, :])
            RW.append(rwt)
        RBB = wpool.tile([P, E], F32, tag="rbb")
        nc.sync.dma_start(RBB[:], rb_d[0:1, :].to_broadcast([P, E]))
        ident = wpool.tile([P, P], F32, tag="ident")
        make_identity(nc, ident[:])
        spT = wpool.tile([E, T], BF16, tag="spt")

        spd = dpool.tile([E, T], BF16, tag="spd")

        # ---------------- router ----------------
        with tc.tile_pool(name="rpsum", bufs=2, space="PSUM") as rpsum, \
             tc.tile_pool(name="tpsum", bufs=2, space="PSUM") as tpsum, \
             tc.tile_pool(name="rsb", bufs=3) as rsb:
            for tt in range(NTT):
                cols = slice(tt * P, (tt + 1) * P)
                pl = rpsum.tile([P, E], F32, tag="rpsum")
                for o in range(OT):
                    nc.tensor.matmul(pl[:], lhsT=Xf[o][:, cols], rhs=RW[o][:],
                                     start=(o == 0), stop=(o == OT - 1))
                lg = rsb.tile([P, E], F32, tag="lg")
                nc.vector.tensor_add(lg[:], pl[:], RBB[:])
                mx = rsb.tile([P, 1], F32, tag="mx")
                nc.vector.reduce_max(mx[:], lg[:], axis=mybir.AxisListType.X)
                nmx = rsb.tile([P, 1], F32, tag="nmx")
                nc.vector.tensor_scalar_mul(nmx[:], mx[:], -1.0)
                ex = rsb.tile([P, E], F32, tag="ex")
                den = rsb.tile([P, 1], F32, tag="den")
                nc.scalar.activation(ex[:], lg[:],
                                     mybir.ActivationFunctionType.Exp,
                                     bias=nmx[:, 0:1], scale=1.0,
                                     accum_out=den[:, 0:1])
                rden = rsb.tile([P, 1], F32, tag="rden")
                nc.vector.reciprocal(rden[:], den[:])
                t8 = rsb.tile([P, 8], F32, tag="t8")
                nc.vector.max(out=t8[:], in_=lg[:])
                mask = rsb.tile([P, E], F32, tag="mask")
                nc.vector.tensor_scalar(mask[:], lg[:], t8[:, 3:4], None,
                                        op0=mybir.AluOpType.is_ge)
                spm = rsb.tile([P, E], F32, tag="spm")
                nc.vector.tensor_mul(spm[:], ex[:], mask[:])
                spv = rsb.tile([P, E], F32, tag="spv")
                nc.vector.tensor_scalar(spv[:], spm[:], rden[:, 0:1], None,
                                        op0=mybir.AluOpType.mult)
                pt = tpsum.tile([E, P], F32, tag="tpsum")
                nc.tensor.transpose(pt[:], spv[:], ident[:])
                nc.vector.tensor_copy(spT[:, cols], pt[:])
                # flush this chunk's gate rows to DRAM for broadcast loads
                if (tt + 1) % TILES_PER_CH == 0:
                    tci = tt // TILES_PER_CH
                    ccols = slice(tci * TCH, (tci + 1) * TCH)
                    nc.sync.dma_start(spd[:, ccols], spT[:, ccols])

        # ---------------- main expert accumulation ----------------
        with tc.tile_pool(name="mpsum", bufs=2, space="PSUM") as mpsum, \
             tc.tile_pool(name="srep", bufs=3) as srpool, \
             tc.tile_pool(name="rs", bufs=2) as rspool, \
             tc.tile_pool(name="outb", bufs=3) as opool:
            for tci in range(NTC):
                ccols = slice(tci * TCH, (tci + 1) * TCH)
                ps = [mpsum.tile([P, TCH], F32, tag=f"ps{a}",
                                 name=f"ps{a}_{tci}") for a in range(4)]
                for e in range(E):
                    srep = srpool.tile([P, TCH], BF16, tag="srep")
                    nc.sync.dma_start(srep[:],
                                      spd[e:e + 1, ccols].to_broadcast([P, TCH]))
                    rs = []
                    for o in range(OT):
                        r = rspool.tile([P, TCH], BF16, tag=f"rs{o}")
                        nc.vector.tensor_mul(r[:], X[o][:, ccols], srep[:])
                        rs.append(r)
                    for a in range(4):
                        for o in range(OT):
                            nc.tensor.matmul(
                                ps[a][:],
                                lhsT=W[o][:, e * A2 + a * P: e * A2 + (a + 1) * P],
                                rhs=rs[o][:],
                                start=(e == 0 and o == 0),
                                stop=False,
                            )
                for a in range(4):
                    nc.tensor.matmul(ps[a][:], lhsT=Bc[:, a * P:(a + 1) * P],
                                     rhs=spT[:, ccols], start=False, stop=True)
                for a in range(2):
                    ob = opool.tile([P, TCH], F32, tag="ob")
                    nc.scalar.copy(ob[:], ps[a][:])
                    nc.sync.dma_start(outT_d[a * P:(a + 1) * P, ccols], ob[:])
                for a in range(2, 4):
                    th = opool.tile([P, TCH], F32, tag="th")
                    nc.scalar.activation(th[:], ps[a][:],
                                         mybir.ActivationFunctionType.Tanh)
                    ob = opool.tile([P, TCH], F32, tag="ob")
                    nc.vector.tensor_scalar(ob[:], th[:], LOG_STD_SCALE,
                                            LOG_STD_SHIFT,
                                            op0=mybir.AluOpType.mult,
                                            op1=mybir.AluOpType.add)
                    nc.sync.dma_start(outT_d[a * P:(a + 1) * P, ccols], ob[:])

    nc.compile()
    return nc


def _host_prep(inputs, ncores=NCORES):
    x = np.asarray(inputs["x"], np.float32)
    rw = np.asarray(inputs["router_w"], np.float32)
    rb = np.asarray(inputs["router_b"], np.float32)
    mw = np.asarray(inputs["mean_w"], np.float32)
    mb = np.asarray(inputs["mean_b"], np.float32)
    lw = np.asarray(inputs["log_std_w"], np.float32)
    lb = np.asarray(inputs["log_std_b"], np.float32)

    B = x.shape[0]
    T = B // ncores

    # wcat[o, e*A2 + a] = (mean|log_std)_w[e, a, o]
    wc = np.concatenate([mw.transpose(0, 2, 1), lw.transpose(0, 2, 1)], axis=2)
    wcat = np.ascontiguousarray(wc.transpose(1, 0, 2)).reshape(OBS, E * A2)
    wcat = wcat.astype(NP_BF16)
    bcat = np.concatenate([mb, lb], axis=1).astype(NP_BF16)
    rwT = np.ascontiguousarray(rw.T).astype(np.float32)
    rbv = rb.reshape(1, E).astype(np.float32)

    shards = x.reshape(ncores, T, OBS)
    in_maps = []
    for c in range(ncores):
        xTf = np.ascontiguousarray(shards[c].T.astype(np.float32))
        in_maps.append({"xT": xTf.astype(NP_BF16), "xTf": xTf,
                        "wcat": wcat, "bcat": bcat,
                        "rwT": rwT, "rb": rbv})
    return in_maps, T


_NC_CACHE = {}


def _get_nc(T):
    if T not in _NC_CACHE:
        _NC_CACHE[T] = build_nc(T)
    return _NC_CACHE[T]


def run_sharded(inputs, trace=False):
    """Returns ((mean, log_std), BassKernelResults)."""
    in_maps, T = _host_prep(inputs)
    nc = _get_nc(T)
    res = run_bass_kernel_spmd(nc, in_maps, list(range(NCORES)), trace=trace)
    outs = [res.results[c]["outT"] for c in range(NCORES)]
    mean = np.concatenate([o[:ACT_DIM].T for o in outs], axis=0)
    log_std = np.concatenate([o[ACT_DIM:].T for o in outs], axis=0)
    return (np.ascontiguousarray(mean, dtype=np.float32),
            np.ascontiguousarray(log_std, dtype=np.float32)), res


def kernel(**inputs):
    (mean, log_std), _ = run_sharded(inputs, trace=False)
    return mean, log_std


# revision 8
# speedup vs baseline: 1.0767x; 1.0767x over previous
"""MoE actor (16 experts, top-4) Trainium2 kernel, data-parallel over 8 NeuronCores.

Math per token t:
    logits = x @ router_w.T + router_b             [E]
    probs  = softmax(logits)
    sp     = probs * topk4_mask(logits)            [E]  (masked, not renormalized)
    mean   = sum_e sp[e] * (x @ mean_w[e].T    + mean_b[e])
    lstd   = sum_e sp[e] * (x @ log_std_w[e].T + log_std_b[e])
    lstd   = 1.75 * tanh(lstd) - 3.25

Device strategy (per core, T=2048 tokens):
  - x arrives transposed+bf16 (xT [512, T]); expert weights arrive as one
    concatenated stack wcat[o, e*512+a] (mean|log_std along a, 512 wide).
  - Router: 64 small matmuls -> logits [t,16]; DVE max8 threshold for top-4
    mask; ACT exp with accumulated denominator; sp -> PE-transpose -> spT.
  - spT bounces through DRAM so it can be partition-broadcast-loaded.
  - Main: for each 512-token chunk, for each expert: scale xT tiles by the
    broadcast gate row (DVE), then 16 bf16 matmuls accumulate all experts +
    bias matmul into 4 PSUM banks = outT[512, chunk] (f32).
  - a-rows 256..511 are log_std: tanh (ACT) + affine (DVE) before store.

No collectives: pure SPMD data parallelism; host shards/gathers.
"""

from contextlib import ExitStack

import ml_dtypes
import numpy as np

import concourse.bass as bass
import concourse.mybir as mybir
import concourse.tile as tile
from concourse import bacc
from concourse.bass_utils import run_bass_kernel_spmd
from concourse.masks import make_identity

BF16 = mybir.dt.bfloat16
F32 = mybir.dt.float32
NP_BF16 = ml_dtypes.bfloat16

P = 128
NCORES = 8
B_FULL = 16384
OBS = 512
ACT_DIM = 256
E = 16
A2 = 2 * ACT_DIM  # 512: mean|log_std concatenated
OT = OBS // P     # 4 o-tiles

LOG_STD_SCALE = 3.5   # 0.5*(LOG_STD_MAX-LOG_STD_MIN)
LOG_STD_SHIFT = -1.5  # LOG_STD_MIN + 0.5*(MAX-MIN)


def build_nc(T):
    """Build the single-core Bacc program for a T-token shard."""
    TCH = min(512, T)       # token chunk (psum free dim)
    NTC = T // TCH          # chunks
    NTT = T // P            # router token tiles
    assert T % P == 0 and (T % TCH == 0)
    TILES_PER_CH = TCH // P

    nc = bacc.Bacc("TRN2", target_bir_lowering=False, debug=False)

    xT_d = nc.declare_dram_parameter("xT", [OBS, T], BF16, isOutput=False)
    xTf_d = nc.declare_dram_parameter("xTf", [OBS, T], F32, isOutput=False)
    wcat_d = nc.declare_dram_parameter("wcat", [OBS, E * A2], BF16, isOutput=False)
    bcat_d = nc.declare_dram_parameter("bcat", [E, A2], BF16, isOutput=False)
    rwT_d = nc.declare_dram_parameter("rwT", [OBS, E], F32, isOutput=False)
    rb_d = nc.declare_dram_parameter("rb", [1, E], F32, isOutput=False)
    outT_d = nc.declare_dram_parameter("outT", [A2, T], F32, isOutput=True)

    with tile.TileContext(nc) as tc, ExitStack() as ctx:
        wpool = ctx.enter_context(tc.tile_pool(name="weights", bufs=1))
        dpool = ctx.enter_context(tc.tile_pool(name="spd", bufs=1, space="DRAM"))
        rpsum = ctx.enter_context(tc.tile_pool(name="rpsum", bufs=2, space="PSUM"))
        tpsum = ctx.enter_context(tc.tile_pool(name="tpsum", bufs=2, space="PSUM"))
        rsb = ctx.enter_context(tc.tile_pool(name="rsb", bufs=3))
        mpsum = ctx.enter_context(tc.tile_pool(name="mpsum", bufs=1, space="PSUM"))
        srpool = ctx.enter_context(tc.tile_pool(name="srep", bufs=2 * E + 2))
        rspool = ctx.enter_context(tc.tile_pool(name="rs", bufs=8))
        opool = ctx.enter_context(tc.tile_pool(name="outb", bufs=3))

        # --- loads, in router-first order so PE can start within ~3us ---
        RW = []
        for o in range(OT):
            rwt = wpool.tile([P, E], F32, tag=f"rw{o}")
            nc.sync.dma_start(rwt[:], rwT_d[o * P:(o + 1) * P, :])
            RW.append(rwt)
        RBB = wpool.tile([P, E], F32, tag="rbb")
        nc.sync.dma_start(RBB[:], rb_d[0:1, :].to_broadcast([P, E]))
        ident = wpool.tile([P, P], F32, tag="ident")
        make_identity(nc, ident[:])
        Bc = wpool.tile([E, A2], BF16, tag="bc")
        nc.sync.dma_start(Bc[:], bcat_d[:, :])

        # x (f32 router copy) chunked by token-chunk so chunk 0 lands first
        Xf = [[None] * NTC for _ in range(OT)]
        for tci in range(NTC):
            for o in range(OT):
                t = wpool.tile([P, TCH], F32, tag=f"xf{o}_{tci}",
                               name=f"xf{o}_{tci}")
                nc.sync.dma_start(
                    t[:], xTf_d[o * P:(o + 1) * P, tci * TCH:(tci + 1) * TCH])
                Xf[o][tci] = t
        W = []
        for o in range(OT):
            wt = wpool.tile([P, E * A2], BF16, tag=f"w{o}")
            nc.sync.dma_start(wt[:], wcat_d[o * P:(o + 1) * P, :])
            W.append(wt)
        X = [[None] * NTC for _ in range(OT)]
        for tci in range(NTC):
            for o in range(OT):
                t = wpool.tile([P, TCH], BF16, tag=f"x{o}_{tci}",
                               name=f"x{o}_{tci}")
                nc.sync.dma_start(
                    t[:], xT_d[o * P:(o + 1) * P, tci * TCH:(tci + 1) * TCH])
                X[o][tci] = t

        spT = wpool.tile([E, T], BF16, tag="spt")
        spd = dpool.tile([E, T], BF16, tag="spd")

        for tci in range(NTC):
            ccols = slice(tci * TCH, (tci + 1) * TCH)
            # ---------------- router for this chunk ----------------
            for lt in range(TILES_PER_CH):
                tt = tci * TILES_PER_CH + lt
                cols = slice(tt * P, (tt + 1) * P)
                lcols = slice(lt * P, (lt + 1) * P)
                pl = rpsum.tile([P, E], F32, tag="rpsum")
                for o in range(OT):
                    nc.tensor.matmul(pl[:], lhsT=Xf[o][tci][:, lcols],
                                     rhs=RW[o][:],
                                     start=(o == 0), stop=(o == OT - 1))
                lg = rsb.tile([P, E], F32, tag="lg")
                nc.vector.tensor_add(lg[:], pl[:], RBB[:])
                mx = rsb.tile([P, 1], F32, tag="mx")
                nc.vector.reduce_max(mx[:], lg[:], axis=mybir.AxisListType.X)
                nmx = rsb.tile([P, 1], F32, tag="nmx")
                nc.vector.tensor_scalar_mul(nmx[:], mx[:], -1.0)
                ex = rsb.tile([P, E], F32, tag="ex")
                den = rsb.tile([P, 1], F32, tag="den")
                nc.scalar.activation(ex[:], lg[:],
                                     mybir.ActivationFunctionType.Exp,
                                     bias=nmx[:, 0:1], scale=1.0,
                                     accum_out=den[:, 0:1])
                rden = rsb.tile([P, 1], F32, tag="rden")
                nc.vector.reciprocal(rden[:], den[:])
                t8 = rsb.tile([P, 8], F32, tag="t8")
                nc.vector.max(out=t8[:], in_=lg[:])
                mask = rsb.tile([P, E], F32, tag="mask")
                nc.vector.tensor_scalar(mask[:], lg[:], t8[:, 3:4], None,
                                        op0=mybir.AluOpType.is_ge)
                spm = rsb.tile([P, E], F32, tag="spm")
                nc.vector.tensor_mul(spm[:], ex[:], mask[:])
                spv = rsb.tile([P, E], F32, tag="spv")
                nc.vector.tensor_scalar(spv[:], spm[:], rden[:, 0:1], None,
                                        op0=mybir.AluOpType.mult)
                pt = tpsum.tile([E, P], F32, tag="tpsum")
                nc.tensor.transpose(pt[:], spv[:], ident[:])
                nc.vector.tensor_copy(spT[:, cols], pt[:])
            # gate rows to DRAM, then broadcast-load one [P, TCH] row/expert
            nc.sync.dma_start(spd[:, ccols], spT[:, ccols])
            sreps = []
            for e in range(E):
                srep = srpool.tile([P, TCH], BF16, tag="srep",
                                   name=f"srep{e}_{tci}")
                nc.sync.dma_start(srep[:],
                                  spd[e:e + 1, ccols].to_broadcast([P, TCH]))
                sreps.append(srep)

            # ---------------- expert accumulation (o-outer) ----------------
            ps = [mpsum.tile([P, TCH], F32, tag=f"ps{a}",
                             name=f"ps{a}_{tci}") for a in range(4)]
            for o in range(OT):
                for e in range(E):
                    r = rspool.tile([P, TCH], BF16, tag="rs",
                                    name=f"rs{o}_{e}_{tci}")
                    nc.vector.tensor_mul(r[:], X[o][tci][:], sreps[e][:])
                    for a in range(4):
                        nc.tensor.matmul(
                            ps[a][:],
                            lhsT=W[o][:, e * A2 + a * P: e * A2 + (a + 1) * P],
                            rhs=r[:],
                            start=(o == 0 and e == 0),
                            stop=False,
                        )
            for a in range(4):
                nc.tensor.matmul(ps[a][:], lhsT=Bc[:, a * P:(a + 1) * P],
                                 rhs=spT[:, ccols], start=False, stop=True)
            for a in range(2):
                ob = opool.tile([P, TCH], F32, tag="ob")
                nc.scalar.copy(ob[:], ps[a][:])
                nc.sync.dma_start(outT_d[a * P:(a + 1) * P, ccols], ob[:])
            for a in range(2, 4):
                th = opool.tile([P, TCH], F32, tag="th")
                nc.scalar.activation(th[:], ps[a][:],
                                     mybir.ActivationFunctionType.Tanh)
                ob = opool.tile([P, TCH], F32, tag="ob")
                nc.vector.tensor_scalar(ob[:], th[:], LOG_STD_SCALE,
                                        LOG_STD_SHIFT,
                                        op0=mybir.AluOpType.mult,
                                        op1=mybir.AluOpType.add)
                nc.sync.dma_start(outT_d[a * P:(a + 1) * P, ccols], ob[:])

    nc.compile()
    return nc


def _host_prep(inputs, ncores=NCORES):
    x = np.asarray(inputs["x"], np.float32)
    rw = np.asarray(inputs["router_w"], np.float32)
    rb = np.asarray(inputs["router_b"], np.float32)
    mw = np.asarray(inputs["mean_w"], np.float32)
    mb = np.asarray(inputs["mean_b"], np.float32)
    lw = np.asarray(inputs["log_std_w"], np.float32)
    lb = np.asarray(inputs["log_std_b"], np.float32)

    B = x.shape[0]
    T = B // ncores

    # wcat[o, e*A2 + a] = (mean|log_std)_w[e, a, o]
    wc = np.concatenate([mw.transpose(0, 2, 1), lw.transpose(0, 2, 1)], axis=2)
    wcat = np.ascontiguousarray(wc.transpose(1, 0, 2)).reshape(OBS, E * A2)
    wcat = wcat.astype(NP_BF16)
    bcat = np.concatenate([mb, lb], axis=1).astype(NP_BF16)
    rwT = np.ascontiguousarray(rw.T).astype(np.float32)
    rbv = rb.reshape(1, E).astype(np.float32)

    shards = x.reshape(ncores, T, OBS)
    in_maps = []
    for c in range(ncores):
        xTf = np.ascontiguousarray(shards[c].T.astype(np.float32))
        in_maps.append({"xT": xTf.astype(NP_BF16), "xTf": xTf,
                        "wcat": wcat, "bcat": bcat,
                        "rwT": rwT, "rb": rbv})
    return in_maps, T


_NC_CACHE = {}


def _get_nc(T):
    if T not in _NC_CACHE:
        _NC_CACHE[T] = build_nc(T)
    return _NC_CACHE[T]


def run_sharded(inputs, trace=False):
    """Returns ((mean, log_std), BassKernelResults)."""
    in_maps, T = _host_prep(inputs)
    nc = _get_nc(T)
    res = run_bass_kernel_spmd(nc, in_maps, list(range(NCORES)), trace=trace)
    outs = [res.results[c]["outT"] for c in range(NCORES)]
    mean = np.concatenate([o[:ACT_DIM].T for o in outs], axis=0)
    log_std = np.concatenate([o[ACT_DIM:].T for o in outs], axis=0)
    return (np.ascontiguousarray(mean, dtype=np.float32),
            np.ascontiguousarray(log_std, dtype=np.float32)), res


def kernel(**inputs):
    (mean, log_std), _ = run_sharded(inputs, trace=False)
    return mean, log_std


# revision 15
# speedup vs baseline: 1.0919x; 1.0142x over previous
"""MoE actor (16 experts, top-4) Trainium2 kernel, data-parallel over 8 NeuronCores.

Math per token t:
    logits = x @ router_w.T + router_b             [E]
    probs  = softmax(logits)
    sp     = probs * topk4_mask(logits)            [E]  (masked, not renormalized)
    mean   = sum_e sp[e] * (x @ mean_w[e].T    + mean_b[e])
    lstd   = sum_e sp[e] * (x @ log_std_w[e].T + log_std_b[e])
    lstd   = 1.75 * tanh(lstd) - 3.25

Device strategy (per core, T=2048 tokens):
  - x arrives transposed+bf16 (xT [512, T]); expert weights arrive as one
    concatenated stack wcat[o, e*512+a] (mean|log_std along a, 512 wide).
  - Router: 64 small matmuls -> logits [t,16]; DVE max8 threshold for top-4
    mask; ACT exp with accumulated denominator; sp -> PE-transpose -> spT.
  - spT bounces through DRAM so it can be partition-broadcast-loaded.
  - Main: for each 512-token chunk, for each expert: scale xT tiles by the
    broadcast gate row (DVE), then 16 bf16 matmuls accumulate all experts +
    bias matmul into 4 PSUM banks = outT[512, chunk] (f32).
  - a-rows 256..511 are log_std: tanh (ACT) + affine (DVE) before store.

No collectives: pure SPMD data parallelism; host shards/gathers.
"""

from contextlib import ExitStack

import ml_dtypes
import numpy as np

import concourse.bass as bass
import concourse.mybir as mybir
import concourse.tile as tile
from concourse import bacc
from concourse.bass_utils import run_bass_kernel_spmd
from concourse.masks import make_identity

BF16 = mybir.dt.bfloat16
F32 = mybir.dt.float32
NP_BF16 = ml_dtypes.bfloat16

P = 128
NCORES = 8
B_FULL = 16384
OBS = 512
ACT_DIM = 256
E = 16
A2 = 2 * ACT_DIM  # 512: mean|log_std concatenated
OT = OBS // P     # 4 o-tiles

LOG_STD_SCALE = 3.5   # 0.5*(LOG_STD_MAX-LOG_STD_MIN)
LOG_STD_SHIFT = -1.5  # LOG_STD_MIN + 0.5*(MAX-MIN)


def build_nc(T):
    """Build the single-core Bacc program for a T-token shard."""
    TCH = min(512, T)       # token chunk (psum free dim)
    NTC = T // TCH          # chunks
    NTT = T // P            # router token tiles
    assert T % P == 0 and (T % TCH == 0)
    TILES_PER_CH = TCH // P

    nc = bacc.Bacc("TRN2", target_bir_lowering=False, debug=False)

    xTf_d = nc.declare_dram_parameter("xTf", [OBS, T], F32, isOutput=False)
    wcat_d = nc.declare_dram_parameter("wcat", [OBS, E * A2], BF16, isOutput=False)
    bcat_d = nc.declare_dram_parameter("bcat", [E, A2], BF16, isOutput=False)
    rwT_d = nc.declare_dram_parameter("rwT", [OBS, E], F32, isOutput=False)
    rb_d = nc.declare_dram_parameter("rb", [1, E], F32, isOutput=False)
    outT_d = nc.declare_dram_parameter("outT", [A2, T], F32, isOutput=True)

    with tile.TileContext(nc) as tc, ExitStack() as ctx:
        wpool = ctx.enter_context(tc.tile_pool(name="weights", bufs=1))
        dpool = ctx.enter_context(tc.tile_pool(name="spd", bufs=1, space="DRAM"))
        rpsum = ctx.enter_context(tc.tile_pool(name="rpsum", bufs=2, space="PSUM"))
        tpsum = ctx.enter_context(tc.tile_pool(name="tpsum", bufs=2, space="PSUM"))
        rsb = ctx.enter_context(tc.tile_pool(name="rsb", bufs=3))
        mpsum = ctx.enter_context(tc.tile_pool(name="mpsum", bufs=1, space="PSUM"))
        srpool = ctx.enter_context(tc.tile_pool(name="srep", bufs=2 * E + 2))
        rspool = ctx.enter_context(tc.tile_pool(name="rs", bufs=8))
        opool = ctx.enter_context(tc.tile_pool(name="outb", bufs=3))

        # --- loads, in router-first order so PE can start within ~3us ---
        RW = []
        for o in range(OT):
            rwt = wpool.tile([P, E], F32, tag=f"rw{o}")
            nc.sync.dma_start(rwt[:], rwT_d[o * P:(o + 1) * P, :])
            RW.append(rwt)
        # x (f32) chunked by token-chunk so chunk 0 lands first
        Xf = [[None] * NTC for _ in range(OT)]
        for tci in range(NTC):
            for o in range(OT):
                t = wpool.tile([P, TCH], F32, tag=f"xf{o}_{tci}",
                               name=f"xf{o}_{tci}")
                nc.sync.dma_start(
                    t[:], xTf_d[o * P:(o + 1) * P, tci * TCH:(tci + 1) * TCH])
                Xf[o][tci] = t
        RBB = wpool.tile([P, E], F32, tag="rbb")
        nc.sync.dma_start(RBB[:], rb_d[0:1, :].to_broadcast([P, E]))
        ident = wpool.tile([P, P], F32, tag="ident")
        make_identity(nc, ident[:])
        Bc = wpool.tile([E, A2], BF16, tag="bc")
        nc.sync.dma_start(Bc[:], bcat_d[:, :])
        W = []
        for o in range(OT):
            wt = wpool.tile([P, E * A2], BF16, tag=f"w{o}")
            nc.sync.dma_start(wt[:], wcat_d[o * P:(o + 1) * P, :])
            W.append(wt)
        # bf16 x via on-device cast
        X = [[None] * NTC for _ in range(OT)]
        for tci in range(NTC):
            for o in range(OT):
                t = wpool.tile([P, TCH], BF16, tag=f"x{o}_{tci}",
                               name=f"x{o}_{tci}")
                nc.vector.tensor_copy(t[:], Xf[o][tci][:])
                X[o][tci] = t

        spT = wpool.tile([E, T], BF16, tag="spt")
        spd = dpool.tile([E, T], BF16, tag="spd")

        LE = TILES_PER_CH * E  # 64 logits columns per chunk (lt-major)

        for tci in range(NTC):
            ccols = slice(tci * TCH, (tci + 1) * TCH)
            # ------- router for this chunk, batched over its 4 token tiles --
            pl = rpsum.tile([P, LE], F32, tag="rpsum")
            for lt in range(TILES_PER_CH):
                lcols = slice(lt * P, (lt + 1) * P)
                for o in range(OT):
                    nc.tensor.matmul(pl[:, lt * E:(lt + 1) * E],
                                     lhsT=Xf[o][tci][:, lcols], rhs=RW[o][:],
                                     start=(o == 0), stop=(o == OT - 1))
            lgb = rsb.tile([P, LE], F32, tag="lgb")
            rbb3 = RBB[:].rearrange("p (l e) -> p l e", l=1).to_broadcast(
                [P, TILES_PER_CH, E])
            nc.vector.tensor_tensor(
                lgb[:].rearrange("p (l e) -> p l e", e=E),
                pl[:].rearrange("p (l e) -> p l e", e=E), rbb3,
                op=mybir.AluOpType.add)
            lg3 = lgb[:].rearrange("p (l e) -> p l e", e=E)
            mx = rsb.tile([P, TILES_PER_CH], F32, tag="mx")
            nc.vector.reduce_max(mx[:], lg3, axis=mybir.AxisListType.X)
            mxb = mx[:].rearrange("p (l e) -> p l e", e=1).to_broadcast(
                [P, TILES_PER_CH, E])
            lgs = rsb.tile([P, LE], F32, tag="lgs")
            nc.vector.tensor_sub(
                lgs[:].rearrange("p (l e) -> p l e", e=E), lg3, mxb)
            ex = rsb.tile([P, LE], F32, tag="ex")
            nc.scalar.activation(ex[:], lgs[:],
                                 mybir.ActivationFunctionType.Exp)
            den = rsb.tile([P, TILES_PER_CH], F32, tag="den")
            nc.vector.reduce_sum(den[:],
                                 ex[:].rearrange("p (l e) -> p l e", e=E),
                                 axis=mybir.AxisListType.X)
            rden = rsb.tile([P, TILES_PER_CH], F32, tag="rden")
            nc.vector.reciprocal(rden[:], den[:])
            t8b = rsb.tile([P, 8 * TILES_PER_CH], F32, tag="t8b")
            for lt in range(TILES_PER_CH):
                nc.vector.max(out=t8b[:, lt * 8:(lt + 1) * 8],
                              in_=lgb[:, lt * E:(lt + 1) * E])
            thrb = t8b[:].rearrange("p (l k) -> p l k", k=8)[:, :, 3:4] \
                .to_broadcast([P, TILES_PER_CH, E])
            mask = rsb.tile([P, LE], F32, tag="mask")
            nc.vector.tensor_tensor(
                mask[:].rearrange("p (l e) -> p l e", e=E), lg3, thrb,
                op=mybir.AluOpType.is_ge)
            spm = rsb.tile([P, LE], F32, tag="spm")
            nc.vector.tensor_mul(spm[:], ex[:], mask[:])
            spv = rsb.tile([P, LE], F32, tag="spv")
            rdenb = rden[:].rearrange("p (l e) -> p l e", e=1).to_broadcast(
                [P, TILES_PER_CH, E])
            nc.vector.tensor_tensor(
                spv[:].rearrange("p (l e) -> p l e", e=E),
                spm[:].rearrange("p (l e) -> p l e", e=E), rdenb,
                op=mybir.AluOpType.mult)
            for lt in range(TILES_PER_CH):
                tt = tci * TILES_PER_CH + lt
                pt = tpsum.tile([E, P], F32, tag="tpsum",
                                name=f"pt{tci}_{lt}")
                nc.tensor.transpose(pt[:], spv[:, lt * E:(lt + 1) * E],
                                    ident[:])
                nc.vector.tensor_copy(spT[:, tt * P:(tt + 1) * P], pt[:])
            # gate rows to DRAM, then broadcast-load one [P, TCH] row/expert
            nc.sync.dma_start(spd[:, ccols], spT[:, ccols])
            sreps = []
            for e in range(E):
                srep = srpool.tile([P, TCH], BF16, tag="srep",
                                   name=f"srep{e}_{tci}")
                nc.sync.dma_start(srep[:],
                                  spd[e:e + 1, ccols].to_broadcast([P, TCH]))
                sreps.append(srep)

            # ---------------- expert accumulation (o-outer) ----------------
            ps = [mpsum.tile([P, TCH], F32, tag=f"ps{a}",
                             name=f"ps{a}_{tci}") for a in range(4)]
            for o in range(OT):
                for e in range(E):
                    r = rspool.tile([P, TCH], BF16, tag="rs",
                                    name=f"rs{o}_{e}_{tci}")
                    nc.vector.tensor_mul(r[:], X[o][tci][:], sreps[e][:])
                    for a in range(4):
                        nc.tensor.matmul(
                            ps[a][:],
                            lhsT=W[o][:, e * A2 + a * P: e * A2 + (a + 1) * P],
                            rhs=r[:],
                            start=(o == 0 and e == 0),
                            stop=False,
                        )
            for a in range(4):
                nc.tensor.matmul(ps[a][:], lhsT=Bc[:, a * P:(a + 1) * P],
                                 rhs=spT[:, ccols], start=False, stop=True)
            for a in range(2):
                ob = opool.tile([P, TCH], F32, tag="ob")
                nc.scalar.copy(ob[:], ps[a][:])
                nc.sync.dma_start(outT_d[a * P:(a + 1) * P, ccols], ob[:])
            for a in range(2, 4):
                th = opool.tile([P, TCH], F32, tag="th")
                nc.scalar.activation(th[:], ps[a][:],
                                     mybir.ActivationFunctionType.Tanh)
                ob = opool.tile([P, TCH], F32, tag="ob")
                nc.vector.tensor_scalar(ob[:], th[:], LOG_STD_SCALE,
                                        LOG_STD_SHIFT,
                                        op0=mybir.AluOpType.mult,
                                        op1=mybir.AluOpType.add)
                nc.sync.dma_start(outT_d[a * P:(a + 1) * P, ccols], ob[:])

    nc.compile()
    return nc


def _host_prep(inputs, ncores=NCORES):
    x = np.asarray(inputs["x"], np.float32)
    rw = np.asarray(inputs["router_w"], np.float32)
    rb = np.asarray(inputs["router_b"], np.float32)
    mw = np.asarray(inputs["mean_w"], np.float32)
    mb = np.asarray(inputs["mean_b"], np.float32)
    lw = np.asarray(inputs["log_std_w"], np.float32)
    lb = np.asarray(inputs["log_std_b"], np.float32)

    B = x.shape[0]
    T = B // ncores

    # wcat[o, e*A2 + a] = (mean|log_std)_w[e, a, o]
    wc = np.concatenate([mw.transpose(0, 2, 1), lw.transpose(0, 2, 1)], axis=2)
    wcat = np.ascontiguousarray(wc.transpose(1, 0, 2)).reshape(OBS, E * A2)
    wcat = wcat.astype(NP_BF16)
    bcat = np.concatenate([mb, lb], axis=1).astype(NP_BF16)
    rwT = np.ascontiguousarray(rw.T).astype(np.float32)
    rbv = rb.reshape(1, E).astype(np.float32)

    shards = x.reshape(ncores, T, OBS)
    in_maps = []
    for c in range(ncores):
        xTf = np.ascontiguousarray(shards[c].T.astype(np.float32))
        in_maps.append({"xTf": xTf, "wcat": wcat, "bcat": bcat,
                        "rwT": rwT, "rb": rbv})
    return in_maps, T


_NC_CACHE = {}


def _get_nc(T):
    if T not in _NC_CACHE:
        _NC_CACHE[T] = build_nc(T)
    return _NC_CACHE[T]


def run_sharded(inputs, trace=False):
    """Returns ((mean, log_std), BassKernelResults)."""
    in_maps, T = _host_prep(inputs)
    nc = _get_nc(T)
    res = run_bass_kernel_spmd(nc, in_maps, list(range(NCORES)), trace=trace)
    outs = [res.results[c]["outT"] for c in range(NCORES)]
    mean = np.concatenate([o[:ACT_DIM].T for o in outs], axis=0)
    log_std = np.concatenate([o[ACT_DIM:].T for o in outs], axis=0)
    return (np.ascontiguousarray(mean, dtype=np.float32),
            np.ascontiguousarray(log_std, dtype=np.float32)), res


def kernel(**inputs):
    (mean, log_std), _ = run_sharded(inputs, trace=False)
    return mean, log_std
